# revision 1
# baseline (speedup 1.0000x reference)
"""Trainium2 Bass kernel for a dense transformer block (B=4, N=2048, C=768, H=12).

Sharding: 8 cores = 4 batches x 2 sequence halves. Each core receives its
batch's rows rolled so its own 1024 query rows are rows 0:1023 (softmax is
permutation-invariant over keys, so key order doesn't matter). Each core
computes LN1 over all 2048 rows, K/V per 4-head group and attention + MLP for
its own 1024 rows, returning a [1024, 768] output slice. No collectives.

All matmuls run in float32r (full PE rate, ~1e-4 rounding). Dataflow stays in
transposed [channel, token] layouts so contractions land on SBUF partitions.
Softmax denominators ride the values matmul as an appended ones-column; the
per-query 1/denom is applied by folding diag(r) into the PE transpose that
restores each head's [channel, token] layout. SBUF is managed as tag-chained
slots (five 24KB/partition slots rotate through the phase-chained tensors).
"""

import numpy as np

B, N, C = 4, 2048, 768
H, DH = 12, 64
HID = 4 * C
SCALE = DH ** -0.5
EPS = 1e-5

P = 128
CT = C // P          # 6
NT = N // P          # 16
NO = N // 2          # 1024 own rows
NOT_ = NO // P       # 8
HT = HID // P        # 24


def _build_bass():
    import concourse.bass as bass
    import concourse.tile as tile
    from concourse import bacc, mybir
    from concourse.masks import make_identity
    from concourse.alu_op_type import AluOpType as A

    F32 = mybir.dt.float32
    F32R = mybir.dt.float32r
    AF = mybir.ActivationFunctionType

    nc = bacc.Bacc("TRN2", target_bir_lowering=False, num_swdge_queues=4)

    xb = nc.dram_tensor("xb", [N, C], F32, kind="ExternalInput")
    w_qkv = nc.dram_tensor("w_qkv", [C, 3 * C], F32, kind="ExternalInput")
    w_proj = nc.dram_tensor("w_proj", [C, C], F32, kind="ExternalInput")
    w_fc1 = nc.dram_tensor("w_fc1", [C, HID], F32, kind="ExternalInput")
    w_fc2 = nc.dram_tensor("w_fc2", [HID, C], F32, kind="ExternalInput")
    ln1_g = nc.dram_tensor("ln1_g", [C], F32, kind="ExternalInput")
    ln1_b = nc.dram_tensor("ln1_b", [C], F32, kind="ExternalInput")
    ln2_g = nc.dram_tensor("ln2_g", [C], F32, kind="ExternalInput")
    ln2_b = nc.dram_tensor("ln2_b", [C], F32, kind="ExternalInput")
    b_proj = nc.dram_tensor("b_proj", [C], F32, kind="ExternalInput")
    b_fc1 = nc.dram_tensor("b_fc1", [HID], F32, kind="ExternalInput")
    b_fc2 = nc.dram_tensor("b_fc2", [C], F32, kind="ExternalInput")
    out = nc.dram_tensor("out", [NO, C], F32, kind="ExternalOutput")

    dma = nc.gpsimd.dma_start

    with tile.TileContext(nc) as tc:
        consts = tc.alloc_tile_pool(name="consts", bufs=1)
        pbc = tc.alloc_tile_pool(name="pbc", bufs=1)        # LN gamma/beta bcast
        psmall = tc.alloc_tile_pool(name="psmall", bufs=1)  # denominators etc.
        work = tc.alloc_tile_pool(name="work", bufs=2)
        main = tc.alloc_tile_pool(name="main", bufs=1)      # five 24KB slots
        stream = tc.alloc_tile_pool(name="stream", bufs=2)  # weights/exp stream
        pkt = tc.alloc_tile_pool(name="pkt", bufs=1)        # K^T per pair

        ident = consts.tile([P, P], F32)
        make_identity(nc, ident)
        ident_r = consts.tile([P, P], F32R)
        nc.vector.tensor_copy(ident_r, ident)
        eps_t = consts.tile([P, 1], F32)
        nc.vector.memset(eps_t, EPS)
        ones_col = consts.tile([P, 1], F32)
        nc.vector.memset(ones_col, 1.0)
        bpT = consts.tile([P, CT], F32)
        dma(out=bpT, in_=b_proj[:].rearrange("(t p) -> p t", p=P))
        bf1T = consts.tile([P, HT], F32)
        dma(out=bf1T, in_=b_fc1[:].rearrange("(t p) -> p t", p=P))
        bf2T = consts.tile([P, CT], F32)
        dma(out=bf2T, in_=b_fc2[:].rearrange("(t p) -> p t", p=P))

        def layernorm_tile(x_t, g_bc, b_bc):
            st = work.tile([P, 3, 6], F32, tag="ln_st")
            for s in range(3):
                nc.vector.bn_stats(out=st[:, s, :], in_=x_t[:, s * 256:(s + 1) * 256])
            mv = work.tile([P, 2], F32, tag="ln_mv")
            nc.vector.bn_aggr(out=mv, in_=st)
            lnv = work.tile([P, 1], F32, tag="ln_lnv")
            nc.scalar.activation(out=lnv, in_=mv[:, 1:2], func=AF.Ln, bias=eps_t)
            r = work.tile([P, 1], F32, tag="ln_r")
            nc.scalar.activation(out=r, in_=lnv, func=AF.Exp, scale=-0.5)
            h = work.tile([P, C], F32, tag="ln_h")
            nc.vector.tensor_scalar(out=h, in0=x_t, scalar1=mv[:, 0:1], scalar2=r,
                                    op0=A.subtract, op1=A.mult)
            nc.vector.tensor_tensor(out=h, in0=h, in1=g_bc, op=A.mult)
            nc.vector.tensor_tensor(out=h, in0=h, in1=b_bc, op=A.add)
            return h

        def transpose_768(src, dst_view, ps_pool, ps_tag="tr"):
            tp = ps_pool.tile([P, C], F32, tag=ps_tag)
            for t in range(CT):
                nc.tensor.transpose(tp[:, t * P:(t + 1) * P],
                                    src[:, t * P:(t + 1) * P], ident)
            nc.vector.tensor_copy(out=dst_view,
                                  in_=tp[:].rearrange("p (t n) -> p t n", t=CT))

        # ---------------- Phase A: LN1 + transpose -> hT0/hT1 [128, 3, 2048] f32r
        hT0 = main.tile([P, 3, N], F32R, tag="S1")
        hT1 = main.tile([P, 3, N], F32R, tag="S2")

        def hts(kt, sl):
            return hT0[:, kt, sl] if kt < 3 else hT1[:, kt - 3, sl]

        g1_bc = pbc.tile([P, C], F32, tag="g_bc")
        dma(out=g1_bc, in_=ln1_g[:].partition_broadcast(P))
        b1_bc = pbc.tile([P, C], F32, tag="b_bc")
        dma(out=b1_bc, in_=ln1_b[:].partition_broadcast(P))
        with tc.tile_pool(name="ps_trA", bufs=2, space="PSUM") as ps_trA:
            for i in range(NT):
                x_t = work.tile([P, C], F32, tag="io")
                dma(out=x_t, in_=xb[i * P:(i + 1) * P, :])
                hg = layernorm_tile(x_t, g1_bc, b1_bc)
                tp = ps_trA.tile([P, C], F32, tag="tr")
                for t in range(CT):
                    nc.tensor.transpose(tp[:, t * P:(t + 1) * P],
                                        hg[:, t * P:(t + 1) * P], ident)
                nc.vector.tensor_copy(
                    out=hT0[:, :, i * P:(i + 1) * P],
                    in_=tp[:, 0:384].rearrange("p (t n) -> p t n", t=3))
                nc.vector.tensor_copy(
                    out=hT1[:, :, i * P:(i + 1) * P],
                    in_=tp[:, 384:768].rearrange("p (t n) -> p t n", t=3))

        # ---------------- Phase B: attention, per group of 4 heads (2 pairs)
        YTraw = main.tile([P, CT, NO], F32, tag="S3")
        den = psmall.tile([H, NO], F32)
        with tc.tile_pool(name="ps_b", bufs=1, space="PSUM") as ps_b, \
             tc.tile_pool(name="ps_y", bufs=1, space="PSUM") as ps_y:
            for pg in range(3):
                # V for heads 4pg..4pg+3, token-major with an appended ones col
                V_g = main.tile([P, NT, 4 * 65], F32R, tag="S5")
                wv = stream.tile([P, CT, 256], F32R, tag="w")
                dma(out=wv, in_=w_qkv[:, 2 * C + 256 * pg:2 * C + 256 * (pg + 1)]
                    .rearrange("(t p) j -> p t j", p=P))
                for i in range(NT):
                    vps = ps_b.tile([P, 256], F32, tag="sA" if i % 2 == 0 else "sB")
                    for kt in range(CT):
                        nc.tensor.matmul(vps, hts(kt, slice(i * P, (i + 1) * P)),
                                         wv[:, kt, :],
                                         start=(kt == 0), stop=(kt == CT - 1))
                    vv = V_g[:, i, :].rearrange("p (h d) -> p h d", h=4)
                    nc.vector.tensor_copy(
                        out=vv[:, :, 0:64],
                        in_=vps[:].rearrange("p (h d) -> p h d", h=4))
                    nc.vector.tensor_copy(out=vv[:, :, 64:65],
                                          in_=ones_col.to_broadcast((P, 4, 1)))
                for pr in range(2):
                    hp = 2 * pg + pr
                    # Q^T (own rows) / K^T (all rows) for this head pair
                    wq = stream.tile([P, CT, P], F32R, tag="w")
                    dma(out=wq, in_=w_qkv[:, hp * P:(hp + 1) * P]
                        .rearrange("(t p) j -> p t j", p=P))
                    qps = ps_b.tile([P, NO], F32, tag="sA")
                    for ch in range(2):
                        for kt in range(CT):
                            nc.tensor.matmul(qps[:, ch * 512:(ch + 1) * 512],
                                             wq[:, kt, :],
                                             hts(kt, slice(ch * 512, (ch + 1) * 512)),
                                             start=(kt == 0), stop=(kt == CT - 1))
                    QT = stream.tile([P, NO], F32R, tag="qt")
                    nc.vector.tensor_copy(QT, qps)
                    wk = stream.tile([P, CT, P], F32R, tag="w")
                    dma(out=wk, in_=w_qkv[:, C + hp * P:C + (hp + 1) * P]
                        .rearrange("(t p) j -> p t j", p=P))
                    KT = pkt.tile([P, N], F32R, tag="kt")
                    for half in range(2):
                        kps = ps_b.tile([P, NO], F32, tag="sA" if half == 0 else "sB")
                        for ch in range(2):
                            c0 = half * NO + ch * 512
                            for kt in range(CT):
                                nc.tensor.matmul(kps[:, ch * 512:(ch + 1) * 512],
                                                 wk[:, kt, :],
                                                 hts(kt, slice(c0, c0 + 512)),
                                                 start=(kt == 0), stop=(kt == CT - 1))
                        nc.vector.tensor_copy(KT[:, half * NO:(half + 1) * NO], kps)

                    yA = ps_y.tile([65, NO], F32, tag="yA")
                    yB = ps_y.tile([65, NO], F32, tag="yB")
                    for m in range(NT):
                        # separate per-head score tiles (separate PSUM banks ->
                        # the two row-group matmuls run concurrently, and exp of
                        # head A overlaps the QK matmuls of head B / tile m+1)
                        spsA = ps_b.tile([P, NO], F32, tag="sA")
                        spsB = ps_b.tile([P, NO], F32, tag="sB")
                        for ch in range(2):
                            nc.tensor.matmul(spsA[:, ch * 512:(ch + 1) * 512],
                                             KT[0:64, m * P:(m + 1) * P],
                                             QT[0:64, ch * 512:(ch + 1) * 512],
                                             start=True, stop=True,
                                             tile_position=(0, 0))
                        for ch in range(2):
                            nc.tensor.matmul(spsB[:, ch * 512:(ch + 1) * 512],
                                             KT[64:128, m * P:(m + 1) * P],
                                             QT[64:128, ch * 512:(ch + 1) * 512],
                                             start=True, stop=True,
                                             tile_position=(64, 0))
                        eA = stream.tile([P, NO], F32R, tag="e")
                        nc.scalar.activation(out=eA, in_=spsA[:], func=AF.Exp,
                                             scale=SCALE)
                        eB = stream.tile([P, NO], F32R, tag="e")
                        nc.scalar.activation(out=eB, in_=spsB[:],
                                             func=AF.Exp, scale=SCALE)
                        for ch in range(2):
                            nc.tensor.matmul(yA[:, ch * 512:(ch + 1) * 512],
                                             V_g[:, m, 65 * 2 * pr:65 * 2 * pr + 65],
                                             eA[:, ch * 512:(ch + 1) * 512],
                                             start=(m == 0), stop=(m == NT - 1))
                        for ch in range(2):
                            nc.tensor.matmul(yB[:, ch * 512:(ch + 1) * 512],
                                             V_g[:, m, 65 * (2 * pr + 1):65 * (2 * pr + 1) + 65],
                                             eB[:, ch * 512:(ch + 1) * 512],
                                             start=(m == 0), stop=(m == NT - 1))
                    # psum -> sbuf; odd head + denominators shift partitions by DMA
                    ytA = stream.tile([65, NO], F32, tag="e")
                    ytB = stream.tile([65, NO], F32, tag="e")
                    nc.vector.tensor_copy(out=YTraw[0:64, hp, :], in_=yA[0:64, :])
                    nc.vector.tensor_copy(out=ytA[64:65, :], in_=yA[64:65, :])
                    nc.vector.tensor_copy(ytB, yB)
                    dma(out=YTraw[64:128, hp, :], in_=ytB[0:64, :])
                    dma(out=den[2 * hp:2 * hp + 1, :], in_=ytA[64:65, :])
                    dma(out=den[2 * hp + 1:2 * hp + 2, :], in_=ytB[64:65, :])

        # ---------------- Phase C: normalize y by 1/den via diag-scaled transposes
        YTn = main.tile([P, CT, NO], F32R, tag="S4")
        rinv = psmall.tile([H, NO], F32R)
        with nc.allow_low_precision(reason="fp32r rounding of softmax denom"):
            nc.vector.reciprocal(out=rinv, in_=den)
        rT = psmall.tile([P, NOT_, H], F32)
        with tc.tile_pool(name="ps_n", bufs=3, space="PSUM") as ps_n:
            for i in range(NOT_):
                rtp = ps_n.tile([P, H], F32, tag="rT", bufs=2)
                nc.tensor.matmul(rtp, rinv[:, i * P:(i + 1) * P], ident_r[0:H, 0:H],
                                 start=True, stop=True)
                nc.vector.tensor_copy(out=rT[:, i, :], in_=rtp)
            for hp in range(CT):
                for i in range(NOT_):
                    # both heads of the pair -> y [token, dim] with tokens on
                    # partitions; 1/den rides the copy as a per-partition scalar
                    ysb2 = work.tile([P, P], F32, tag="ysb2")
                    for sub in range(2):
                        h, lo = 2 * hp + sub, 64 * sub
                        yps = ps_n.tile([P, 64], F32, tag="y_nt")
                        nc.tensor.transpose(yps,
                                            YTraw[lo:lo + 64, hp, i * P:(i + 1) * P],
                                            ident[lo:lo + 64, lo:lo + 64])
                        nc.vector.tensor_scalar(out=ysb2[:, lo:lo + 64], in0=yps,
                                                scalar1=rT[:, i, h:h + 1],
                                                scalar2=None, op0=A.mult)
                    ytp = ps_n.tile([P, P], F32, tag="yT_n")
                    nc.tensor.transpose(ytp, ysb2, ident)
                    nc.vector.tensor_copy(out=YTn[:, hp, i * P:(i + 1) * P],
                                          in_=ytp)

        # ---------------- Phase D: proj -> attnT (S5 slot)
        attnT = main.tile([P, CT, NO], F32, tag="S5")
        with tc.tile_pool(name="ps_p", bufs=4, space="PSUM") as ps_p:
            for cp in range(CT):
                wp = stream.tile([P, CT, P], F32R, tag="w")
                dma(out=wp, in_=w_proj[:, cp * P:(cp + 1) * P]
                    .rearrange("(t p) j -> p t j", p=P))
                pps = ps_p.tile([P, NO], F32, tag="p")
                for ch in range(2):
                    for kt in range(CT):
                        nc.tensor.matmul(pps[:, ch * 512:(ch + 1) * 512],
                                         wp[:, kt, :],
                                         YTn[:, kt, ch * 512:(ch + 1) * 512],
                                         start=(kt == 0), stop=(kt == CT - 1))
                nc.vector.tensor_scalar(out=attnT[:, cp, :], in0=pps,
                                        scalar1=bpT[:, cp:cp + 1], scalar2=None,
                                        op0=A.add)

        # ---------------- Phase E: attn + residual -> x2; LN2 -> x2lnT
        x2 = main.tile([P, NOT_, C], F32, tag="S4")
        x2lnT = main.tile([P, CT, NO], F32R, tag="S3")
        g2_bc = pbc.tile([P, C], F32, tag="g_bc")
        dma(out=g2_bc, in_=ln2_g[:].partition_broadcast(P))
        b2_bc = pbc.tile([P, C], F32, tag="b_bc")
        dma(out=b2_bc, in_=ln2_b[:].partition_broadcast(P))
        with tc.tile_pool(name="ps_trE", bufs=4, space="PSUM") as ps_trE:
            # sweep 1: attn^T -> attn, + residual -> x2 (PE + DVE pipeline)
            for i in range(NOT_):
                tp = ps_trE.tile([P, C], F32, tag="tr")
                for t in range(CT):
                    nc.tensor.transpose(tp[:, t * P:(t + 1) * P],
                                        attnT[:, t, i * P:(i + 1) * P], ident)
                xo = work.tile([P, C], F32, tag="io")
                dma(out=xo, in_=xb[i * P:(i + 1) * P, :])
                nc.vector.tensor_tensor(out=x2[:, i, :], in0=tp, in1=xo, op=A.add)
            # sweep 2: LN2 + transpose -> x2lnT
            for i in range(NOT_):
                hg2 = layernorm_tile(x2[:, i, :], g2_bc, b2_bc)
                transpose_768(hg2, x2lnT[:, :, i * P:(i + 1) * P], ps_trE)

        # ---------------- Phase F: MLP + residual + output, per 512-token half.
        # fc2 accumulates into six persistent PSUM banks as each gelu tile is
        # produced, so fc1/gelu/fc2 fully pipeline and no activation buffer is
        # needed in SBUF. w_fc2 row-slices load in natural [hid, c'] layout.
        for nh in range(2):
            sl = slice(nh * 512, (nh + 1) * 512)
            with tc.tile_pool(name="ps_mA%d" % nh, bufs=1, space="PSUM") as ps_mA:
                f2s = [ps_mA.tile([P, 512], F32, tag="f2c%d" % cp,
                                  name="f2acc%d_%d" % (nh, cp))
                       for cp in range(CT)]
                for ht in range(HT):
                    w1 = stream.tile([P, CT, P], F32R, tag="wf1", bufs=2)
                    dma(out=w1, in_=w_fc1[:, ht * P:(ht + 1) * P]
                        .rearrange("(t p) j -> p t j", p=P))
                    w2r = stream.tile([P, C], F32R, tag="wf2", bufs=2)
                    dma(out=w2r, in_=w_fc2[ht * P:(ht + 1) * P, :])
                    fps = ps_mA.tile([P, 512], F32,
                                     tag="f1a" if ht % 2 == 0 else "f1b")
                    for kt in range(CT):
                        nc.tensor.matmul(fps, w1[:, kt, :], x2lnT[:, kt, sl],
                                         start=(kt == 0), stop=(kt == CT - 1))
                    ga = work.tile([P, 512], F32R, tag="ga", bufs=3)
                    nc.scalar.activation(out=ga, in_=fps[:], func=AF.Gelu,
                                         bias=bf1T[:, ht:ht + 1])
                    for cp in range(CT):
                        nc.tensor.matmul(f2s[cp], w2r[:, cp * P:(cp + 1) * P], ga,
                                         start=(ht == 0), stop=(ht == HT - 1))
                mlpT = main.tile([P, CT, 512], F32, tag="S5")
                for cp in range(CT):
                    nc.vector.tensor_scalar(out=mlpT[:, cp, :], in0=f2s[cp],
                                            scalar1=bf2T[:, cp:cp + 1],
                                            scalar2=None, op0=A.add)
            with tc.tile_pool(name="ps_o%d" % nh, bufs=2, space="PSUM") as ps_o:
                for i in range(4):
                    it = nh * 4 + i
                    tp = ps_o.tile([P, C], F32, tag="tr")
                    for t in range(CT):
                        nc.tensor.transpose(tp[:, t * P:(t + 1) * P],
                                            mlpT[:, t, i * P:(i + 1) * P], ident)
                    o_sb = work.tile([P, C], F32, tag="io")
                    nc.vector.tensor_tensor(out=o_sb, in0=tp, in1=x2[:, it, :],
                                            op=A.add)
                    dma(out=out[it * P:(it + 1) * P, :], in_=o_sb)

        pkt.release()
        stream.release()
        main.release()
        work.release()
        psmall.release()
        pbc.release()
        consts.release()

    nc.compile()
    return nc


_NC_CACHE = None


def kernel(x, ln1_g, ln1_b, w_qkv, w_proj, b_proj, ln2_g, ln2_b,
           w_fc1, b_fc1, w_fc2, b_fc2):
    global _NC_CACHE
    from concourse.bass_utils import run_bass_kernel_spmd

    x = np.asarray(x, dtype=np.float32)
    shared = {
        "w_qkv": np.asarray(w_qkv, np.float32),
        "w_proj": np.asarray(w_proj, np.float32),
        "w_fc1": np.asarray(w_fc1, np.float32),
        "w_fc2": np.asarray(w_fc2, np.float32),
        "ln1_g": np.asarray(ln1_g, np.float32),
        "ln1_b": np.asarray(ln1_b, np.float32),
        "ln2_g": np.asarray(ln2_g, np.float32),
        "ln2_b": np.asarray(ln2_b, np.float32),
        "b_proj": np.asarray(b_proj, np.float32),
        "b_fc1": np.asarray(b_fc1, np.float32),
        "b_fc2": np.asarray(b_fc2, np.float32),
    }
    in_maps = []
    for c in range(8):
        b, h = c // 2, c % 2
        xbv = np.ascontiguousarray(np.roll(x[b], -h * NO, axis=0))
        in_maps.append({"xb": xbv, **shared})

    if _NC_CACHE is None:
        _NC_CACHE = _build_bass()
    res = run_bass_kernel_spmd(_NC_CACHE, in_maps, core_ids=list(range(8)))

    outp = np.empty((B, N, C), np.float32)
    for c in range(8):
        b, h = c // 2, c % 2
        outp[b, h * NO:(h + 1) * NO, :] = res.results[c]["out"]
    return outp



# revision 10
# speedup vs baseline: 1.6763x; 1.6763x over previous
"""Trainium2 Bass kernel for a dense transformer block (B=4, N=2048, C=768, H=12).

Sharding: 8 cores = 4 batches x 2 sequence halves; each core's batch rows are
rolled so its own 1024 query rows are rows 0:1023 (softmax is permutation-
invariant over keys). Each core computes LN1 over all 2048 rows, full K/V,
attention + MLP for its own 1024 rows. No collectives.

All GEMMs run in fp8 (e4m3) DoubleRow perf mode: operands are [128, 2, *]
access patterns holding two stacked 128-deep contraction tiles (contraction
index c = p + 128*j + 256*instr), which the cost model streams at 0.5
cycles/output-column (4x the f32r rate). Weights are quantized+packed host-
side with power-of-two scales chosen to center fp8's dynamic range; descales
fold into activation scale arguments and output tensor_scalar copies.
Softmax: exp on ACT writes fp8 probability tiles that feed the attn@V
DoubleRow matmul directly; a 16-valued ones-column appended to V accumulates
the denominator on the same partition as its query row, so normalization is a
per-partition reciprocal+scale. Transposes run in bf16 (PE, 1 cycle/row).
"""

import numpy as np
import ml_dtypes

B, N, C = 4, 2048, 768
H, DH = 12, 64
HID = 4 * C
SCALE = DH ** -0.5
EPS = 1e-5

P = 128
NO = 1024           # own query rows per core
NT = N // P         # 16 token tiles
NOT_ = NO // P      # 8 own token tiles

FP8 = ml_dtypes.float8_e4m3
BF16NP = ml_dtypes.bfloat16

WQ_S = 16.0 * SCALE ** 0.5   # on w_q and w_k (so scores psum = 256*SCALE*qk)
WV_S = 16.0                  # on w_v; ones-column is 16 so the scale cancels
WP_S = 16.0
Y_S = 32.0                   # on normalized attention output
W1_S = 16.0
W2_S = 32.0


def _build_bass(ln1_affine, ln2_affine):
    import concourse.bass as bass
    import concourse.tile as tile
    from concourse import bacc, mybir
    from concourse.masks import make_identity
    from concourse.alu_op_type import AluOpType as A

    F32 = mybir.dt.float32
    F8 = mybir.dt.float8e4
    B16 = mybir.dt.bfloat16
    AF = mybir.ActivationFunctionType
    PM = mybir.MatmulPerfMode
    DR = PM.DoubleRow

    nc = bacc.Bacc("TRN2", target_bir_lowering=False, num_swdge_queues=4)

    xb = nc.dram_tensor("xb", [N, C], F32, kind="ExternalInput")
    wq_p = nc.dram_tensor("wq_p", [P, 3, 2, C], F8, kind="ExternalInput")
    wk_p = nc.dram_tensor("wk_p", [P, 3, 2, C], F8, kind="ExternalInput")
    wv_p = nc.dram_tensor("wv_p", [P, 3, 2, C], F8, kind="ExternalInput")
    wp_p = nc.dram_tensor("wp_p", [P, 3, 2, C], F8, kind="ExternalInput")
    w1_p = nc.dram_tensor("w1_p", [P, 3, 2, HID], F8, kind="ExternalInput")
    w2_p = nc.dram_tensor("w2_p", [P, 12, 2, C], F8, kind="ExternalInput")
    bp_s = nc.dram_tensor("bp_s", [C], F32, kind="ExternalInput")    # 512*b_proj
    bf1_d = nc.dram_tensor("bf1_d", [HID], F32, kind="ExternalInput")
    bf2_s = nc.dram_tensor("bf2_s", [C], F32, kind="ExternalInput")  # 32*b_fc2
    if ln1_affine:
        ln1_g = nc.dram_tensor("ln1_g", [C], F32, kind="ExternalInput")
        ln1_b = nc.dram_tensor("ln1_b", [C], F32, kind="ExternalInput")
    if ln2_affine:
        ln2_g = nc.dram_tensor("ln2_g", [C], F32, kind="ExternalInput")
        ln2_b = nc.dram_tensor("ln2_b", [C], F32, kind="ExternalInput")
    out_d = nc.dram_tensor("out", [NO, C], F32, kind="ExternalOutput")

    dma = nc.gpsimd.dma_start

    with tile.TileContext(nc) as tc:
        consts = tc.alloc_tile_pool(name="consts", bufs=1)
        pers = tc.alloc_tile_pool(name="pers", bufs=1)
        work = tc.alloc_tile_pool(name="work", bufs=2)
        attn = tc.alloc_tile_pool(name="attn", bufs=1)

        identf = consts.tile([P, P], F32)
        make_identity(nc, identf)
        identb = consts.tile([P, P], B16)
        nc.vector.tensor_copy(identb, identf)
        eps_t = consts.tile([P, 1], F32)
        nc.vector.memset(eps_t, EPS)
        bpT = consts.tile([P, 6], F32)
        bf1T = consts.tile([P, 24], F32)
        bf2T = consts.tile([P, 6], F32)

        # ---- persistent tiles
        x2 = pers.tile([P, NOT_, C], F32)
        h2T = pers.tile([P, 6, NO], F8)
        yT = pers.tile([P, 3, 2, NO], F8)
        mvs = pers.tile([P, NT, 2], F32)
        rsig = pers.tile([P, NT], F32)
        mvs2 = pers.tile([P, NOT_, 2], F32)
        rsig2 = pers.tile([P, NOT_], F32)
        w1_t = pers.tile([P, 3, 2, HID], F8)
        w2_t = pers.tile([P, 12, 2, C], F8)

        # ---- attention-phase tiles
        hT = attn.tile([P, 6, N], F8)
        QT = [attn.tile([96, 2, NO], F8, name="QT%d" % g) for g in range(4)]
        KT = [attn.tile([96, 2, N], F8, name="KT%d" % g) for g in range(4)]
        Vp = attn.tile([P, NT // 2, 2, H, 65], F8)
        y_sb = [attn.tile([P, NOT_, DH], B16, name="ysb%d" % h) for h in range(H)]
        attnT = attn.tile([P, 6, NO], B16)
        wq_t = attn.tile([P, 3, 2, C], F8)
        wk_t = attn.tile([P, 3, 2, C], F8)
        wv_t = attn.tile([P, 3, 2, C], F8)
        wp_t = attn.tile([P, 3, 2, C], F8)

        if ln1_affine:
            g1_bc = attn.tile([P, C], F32)
            dma(out=g1_bc, in_=ln1_g[:].partition_broadcast(P))
            b1_bc = attn.tile([P, C], F32)
            dma(out=b1_bc, in_=ln1_b[:].partition_broadcast(P))
        if ln2_affine:
            g2_bc = pers.tile([P, C], F32)
            dma(out=g2_bc, in_=ln2_g[:].partition_broadcast(P))
            b2_bc = pers.tile([P, C], F32)
            dma(out=b2_bc, in_=ln2_b[:].partition_broadcast(P))

        nc.vector.memset(Vp[:, :, :, :, 64:65], 16.0)

        def ln_stats(xt, mvt, i):
            st = work.tile([P, 3, 6], F32, tag="st")
            for s in range(3):
                nc.vector.bn_stats(out=st[:, s, :], in_=xt[:, s * 256:(s + 1) * 256])
            nc.vector.bn_aggr(out=mvt[:, i, :], in_=st)

        def ln_rsqrt(mvt, rst, sl):
            lnv = work.tile([P, 4], F32, tag="lnv")
            n = sl.stop - sl.start
            nc.scalar.activation(out=lnv[:, 0:n], in_=mvt[:, sl, 1], func=AF.Ln,
                                 bias=eps_t)
            nc.scalar.activation(out=rst[:, sl], in_=lnv[:, 0:n], func=AF.Exp,
                                 scale=-0.5)

        def ln_normalize(xt, mvt, rst, i, g_bc, b_bc):
            hb = work.tile([P, C], B16, tag="hb", bufs=2)
            with nc.allow_low_precision(reason="bf16 ln out"):
                nc.vector.tensor_scalar(out=hb, in0=xt,
                                        scalar1=mvt[:, i, 0:1],
                                        scalar2=rst[:, i:i + 1],
                                        op0=A.subtract, op1=A.mult)
                if g_bc is not None:
                    nc.vector.tensor_tensor(out=hb, in0=hb, in1=g_bc, op=A.mult)
                if b_bc is not None:
                    nc.vector.tensor_tensor(out=hb, in0=hb, in1=b_bc, op=A.add)
            return hb

        # =========== Phase A: LN1 (+transpose to fp8 hT) and QKV projections
        def load_x_chunk(c0, ntile, tag="xf"):
            xt = work.tile([P, ntile, C], F32, tag=tag)
            dma(out=xt, in_=xb[c0 * P:(c0 + ntile) * P, :]
                .rearrange("(i p) c -> p i c", p=P))
            return xt

        with tc.tile_pool(name="ps_a", bufs=2, space="PSUM") as ps_a, \
             tc.tile_pool(name="ps_b", bufs=2, space="PSUM") as ps_b:

            def phase_a_chunk(ch):
                xt = load_x_chunk(2 * ch, 2)
                for k in range(2):
                    i = 2 * ch + k
                    ln_stats(xt[:, k, :], mvs, i)
                ln_rsqrt(mvs, rsig, slice(2 * ch, 2 * ch + 2))
                for k in range(2):
                    i = 2 * ch + k
                    hb = ln_normalize(xt[:, k, :], mvs, rsig, i,
                                      g1_bc if ln1_affine else None,
                                      b1_bc if ln1_affine else None)
                    tp = ps_a.tile([P, C], B16, tag="tr")
                    for t in range(6):
                        nc.tensor.transpose(tp[:, t * P:(t + 1) * P],
                                            hb[:, t * P:(t + 1) * P], identb)
                    with nc.allow_low_precision(reason="fp8 hT"):
                        nc.vector.tensor_copy(
                            out=hT[:, :, i * P:(i + 1) * P],
                            in_=tp[:].rearrange("p (t n) -> p t n", t=6))

            phase_a_chunk(0)
            dma(out=wq_t, in_=wq_p[:, :, :, :])
            phase_a_chunk(1)
            dma(out=wk_t, in_=wk_p[:, :, :, :])
            phase_a_chunk(2)
            dma(out=wv_t, in_=wv_p[:, :, :, :])
            phase_a_chunk(3)
            dma(out=wp_t, in_=wp_p[:, :, :, :])

            # Q for own 1024 rows (emitted now; PE waits only on hT[:, :, :1024])
            for g in range(4):
                for dj in range(2):
                    cw = 96 * (2 * g + dj)
                    for ch2 in range(2):
                        qp = ps_b.tile([P, 512], F32, tag="qk")
                        for ci in range(3):
                            nc.tensor.matmul(
                                qp[0:96, :], wq_t[:, ci, :, cw:cw + 96],
                                hT[:, 2 * ci:2 * ci + 2, ch2 * 512:(ch2 + 1) * 512],
                                start=(ci == 0), stop=(ci == 2), perf_mode=DR)
                        with nc.allow_low_precision(reason="fp8 qt"):
                            nc.vector.tensor_copy(
                                out=QT[g][:, dj, ch2 * 512:(ch2 + 1) * 512],
                                in_=qp[0:96, :])

            for ch in range(4, 8):
                phase_a_chunk(ch)
            dma(out=bpT, in_=bp_s[:].rearrange("(t p) -> p t", p=P))
            dma(out=bf1T, in_=bf1_d[:].rearrange("(t p) -> p t", p=P))
            dma(out=bf2T, in_=bf2_s[:].rearrange("(t p) -> p t", p=P))

            # K over all 2048 rows
            for g in range(4):
                for dj in range(2):
                    cw = 96 * (2 * g + dj)
                    for ch4 in range(4):
                        kp = ps_b.tile([P, 512], F32, tag="qk")
                        for ci in range(3):
                            nc.tensor.matmul(
                                kp[0:96, :], wk_t[:, ci, :, cw:cw + 96],
                                hT[:, 2 * ci:2 * ci + 2, ch4 * 512:(ch4 + 1) * 512],
                                start=(ci == 0), stop=(ci == 2), perf_mode=DR)
                        with nc.allow_low_precision(reason="fp8 kt"):
                            nc.vector.tensor_copy(
                                out=KT[g][:, dj, ch4 * 512:(ch4 + 1) * 512],
                                in_=kp[0:96, :])

            # V over all rows, [token, vdim] tiles -> paired Vp layout
            for i in range(NT):
                for pg in range(3):
                    vp = ps_b.tile([P, 256], F32, tag="v")
                    for ci in range(3):
                        nc.tensor.matmul(
                            vp, hT[:, 2 * ci:2 * ci + 2, i * P:(i + 1) * P],
                            wv_t[:, ci, :, 256 * pg:256 * (pg + 1)],
                            start=(ci == 0), stop=(ci == 2), perf_mode=DR)
                    with nc.allow_low_precision(reason="fp8 v"):
                        nc.vector.tensor_copy(
                            out=Vp[:, i // 2, i % 2, 4 * pg:4 * pg + 4, 0:64],
                            in_=vp[:].rearrange("p (h d) -> p h d", h=4))

        # MLP weights arrive during attention
        dma(out=w1_t, in_=w1_p[:, :, :, :])
        dma(out=w2_t, in_=w2_p[:, :, :, :])

        # =========== Phase C: attention per head
        with tc.tile_pool(name="ps_s", bufs=2, space="PSUM") as ps_s, \
             tc.tile_pool(name="ps_y", bufs=1, space="PSUM") as ps_y, \
             tc.tile_pool(name="ps_t", bufs=2, space="PSUM") as ps_t:
            for h in range(H):
                g, p0 = h // 3, 32 * (h % 3)
                ya = [ps_y.tile([P, 4, 65], F32, tag="ya%d" % half,
                                name="ya_%d_%d" % (h, half)) for half in range(2)]
                for mp in range(NT // 2):
                    eA = work.tile([P, 2, NO], F8, tag="eA")
                    for sub in range(2):
                        m = 2 * mp + sub
                        sp = ps_s.tile([P, NO], F32, tag="s")
                        for ch2 in range(2):
                            nc.tensor.matmul(
                                sp[:, ch2 * 512:(ch2 + 1) * 512],
                                KT[g][p0:p0 + 32, :, m * P:(m + 1) * P],
                                QT[g][p0:p0 + 32, :, ch2 * 512:(ch2 + 1) * 512],
                                start=True, stop=True, perf_mode=DR)
                        with nc.allow_low_precision(reason="fp8 probs"):
                            nc.scalar.activation(out=eA[:, sub, :], in_=sp,
                                                 func=AF.Exp, scale=1.0 / 256.0)
                    for qt in range(NOT_):
                        nc.tensor.matmul(
                            ya[qt // 4][:, qt % 4, :],
                            eA[:, :, qt * P:(qt + 1) * P],
                            Vp[:, mp, :, h, :],
                            start=(mp == 0), stop=(mp == NT // 2 - 1),
                            perf_mode=DR)
                rin = work.tile([P, NOT_], F32, tag="rin")
                with nc.allow_low_precision(reason="softmax denom recip"):
                    nc.vector.reciprocal(rin[:, 0:4], ya[0][:, :, 64])
                    nc.vector.reciprocal(rin[:, 4:8], ya[1][:, :, 64])
                for qt in range(NOT_):
                    with nc.allow_low_precision(reason="bf16 y"):
                        nc.vector.tensor_scalar(
                            out=y_sb[h][:, qt, :],
                            in0=ya[qt // 4][:, qt % 4, 0:64],
                            scalar1=rin[:, qt:qt + 1], scalar2=Y_S,
                            op0=A.mult, op1=A.mult)
                # transpose pair of heads into yT once the odd head lands
                if h % 2 == 1:
                    i3, jj = h // 4, (h % 4) // 2
                    for qt in range(NOT_):
                        ytp = ps_t.tile([P, P], B16, tag="yt")
                        nc.tensor.transpose(ytp[0:64, :], y_sb[h - 1][:, qt, :],
                                            identb)
                        nc.tensor.transpose(ytp[64:128, :], y_sb[h][:, qt, :],
                                            identb)
                        with nc.allow_low_precision(reason="fp8 yT"):
                            nc.vector.tensor_copy(
                                out=yT[:, i3, jj, qt * P:(qt + 1) * P], in_=ytp)

        # =========== Phase D: output projection
        with tc.tile_pool(name="ps_p", bufs=2, space="PSUM") as ps_p:
            for jt in range(6):
                for ch2 in range(2):
                    pp = ps_p.tile([P, 512], F32, tag="pp")
                    for ci in range(3):
                        nc.tensor.matmul(
                            pp, wp_t[:, ci, :, jt * P:(jt + 1) * P],
                            yT[:, ci, :, ch2 * 512:(ch2 + 1) * 512],
                            start=(ci == 0), stop=(ci == 2), perf_mode=DR)
                    with nc.allow_low_precision(reason="bf16 attnT"):
                        nc.vector.tensor_scalar(
                            out=attnT[:, jt, ch2 * 512:(ch2 + 1) * 512], in0=pp,
                            scalar1=bpT[:, jt:jt + 1], scalar2=1.0 / 512.0,
                            op0=A.add, op1=A.mult)

        # =========== Phase E: residual + LN2 -> h2T
        with tc.tile_pool(name="ps_e", bufs=3, space="PSUM") as ps_e:
            for i in range(NOT_):
                if i % 2 == 0:
                    xo = load_x_chunk(i, 2, tag="xf")
                tr = ps_e.tile([P, C], B16, tag="tr")
                for t in range(6):
                    nc.tensor.transpose(tr[:, t * P:(t + 1) * P],
                                        attnT[:, t, i * P:(i + 1) * P], identb)
                nc.vector.tensor_tensor(out=x2[:, i, :], in0=tr,
                                        in1=xo[:, i % 2, :], op=A.add)
                ln_stats(x2[:, i, :], mvs2, i)
                if i % 2 == 1:
                    ln_rsqrt(mvs2, rsig2, slice(i - 1, i + 1))
            for i in range(NOT_):
                hb2 = ln_normalize(x2[:, i, :], mvs2, rsig2, i,
                                   g2_bc if ln2_affine else None,
                                   b2_bc if ln2_affine else None)
                tp2 = ps_e.tile([P, C], B16, tag="tr")
                for t in range(6):
                    nc.tensor.transpose(tp2[:, t * P:(t + 1) * P],
                                        hb2[:, t * P:(t + 1) * P], identb)
                with nc.allow_low_precision(reason="fp8 h2T"):
                    nc.vector.tensor_copy(
                        out=h2T[:, :, i * P:(i + 1) * P],
                        in_=tp2[:].rearrange("p (t n) -> p t n", t=6))

        attn.release()

        # =========== Phase F: MLP per 512-token half + output
        for nh in range(2):
            sl = slice(nh * 512, (nh + 1) * 512)
            with tc.tile_pool(name="ps_m%d" % nh, bufs=1, space="PSUM") as ps_m:
                f2s = [ps_m.tile([P, 512], F32, tag="f2c%d" % jt,
                                 name="f2acc%d_%d" % (nh, jt))
                       for jt in range(6)]
                for t2 in range(12):
                    gh = work.tile([P, 2, 512], F8, tag="ghat")
                    for j2 in range(2):
                        fps = ps_m.tile([P, 512], F32, tag="f1", bufs=2)
                        hw = P * (2 * t2 + j2)
                        for ci in range(3):
                            nc.tensor.matmul(
                                fps, w1_t[:, ci, :, hw:hw + P],
                                h2T[:, 2 * ci:2 * ci + 2, sl],
                                start=(ci == 0), stop=(ci == 2), perf_mode=DR)
                        with nc.allow_low_precision(reason="fp8 gelu"):
                            nc.scalar.activation(
                                out=gh[:, j2, :], in_=fps, func=AF.Gelu,
                                scale=1.0 / W1_S,
                                bias=bf1T[:, 2 * t2 + j2:2 * t2 + j2 + 1])
                    for jt in range(6):
                        nc.tensor.matmul(
                            f2s[jt], w2_t[:, t2, :, jt * P:(jt + 1) * P], gh,
                            start=(t2 == 0), stop=(t2 == 11), perf_mode=DR)
                mlpT = work.tile([P, 6, 512], B16, tag="mlpT")
                for jt in range(6):
                    with nc.allow_low_precision(reason="bf16 mlpT"):
                        nc.vector.tensor_scalar(
                            out=mlpT[:, jt, :], in0=f2s[jt],
                            scalar1=bf2T[:, jt:jt + 1], scalar2=1.0 / W2_S,
                            op0=A.add, op1=A.mult)
            with tc.tile_pool(name="ps_o%d" % nh, bufs=2, space="PSUM") as ps_o:
                for it in range(4):
                    i = 4 * nh + it
                    tro = ps_o.tile([P, C], B16, tag="tro")
                    for t in range(6):
                        nc.tensor.transpose(tro[:, t * P:(t + 1) * P],
                                            mlpT[:, t, it * P:(it + 1) * P],
                                            identb)
                    o_sb = work.tile([P, C], F32, tag="osb")
                    nc.vector.tensor_tensor(out=o_sb, in0=tro, in1=x2[:, i, :],
                                            op=A.add)
                    dma(out=out_d[i * P:(i + 1) * P, :], in_=o_sb)

        work.release()
        pers.release()
        consts.release()

    nc.compile()
    return nc


def _pack_dr(w):
    """[C_contract, cols] -> [128, C/256, 2, cols] with c = p + 128*j + 256*i."""
    ctr, cols = w.shape
    return np.ascontiguousarray(
        w.reshape(ctr // 256, 2, 128, cols).transpose(2, 0, 1, 3))


def _reorder_qk(w):
    """Reorder head-dim cols: (tg, h3, dj, p32) -> (tg, dj, h3, p32)."""
    return np.ascontiguousarray(
        w.reshape(C, 4, 3, 2, 32).transpose(0, 1, 3, 2, 4).reshape(C, C))


_NC_CACHE = None
_NC_KEY = None


def kernel(x, ln1_g, ln1_b, w_qkv, w_proj, b_proj, ln2_g, ln2_b,
           w_fc1, b_fc1, w_fc2, b_fc2):
    global _NC_CACHE, _NC_KEY
    from concourse.bass_utils import run_bass_kernel_spmd

    x = np.asarray(x, np.float32)
    w_qkv = np.asarray(w_qkv, np.float32)
    ln1_g = np.asarray(ln1_g, np.float32)
    ln1_b = np.asarray(ln1_b, np.float32)
    ln2_g = np.asarray(ln2_g, np.float32)
    ln2_b = np.asarray(ln2_b, np.float32)

    ln1_affine = not (np.all(ln1_g == 1.0) and np.all(ln1_b == 0.0))
    ln2_affine = not (np.all(ln2_g == 1.0) and np.all(ln2_b == 0.0))

    wq = _reorder_qk(w_qkv[:, 0:C]) * WQ_S
    wk = _reorder_qk(w_qkv[:, C:2 * C]) * WQ_S
    wv = w_qkv[:, 2 * C:3 * C] * WV_S

    shared = {
        "wq_p": _pack_dr(wq).astype(FP8),
        "wk_p": _pack_dr(wk).astype(FP8),
        "wv_p": _pack_dr(wv).astype(FP8),
        "wp_p": _pack_dr(np.asarray(w_proj, np.float32) * WP_S).astype(FP8),
        "w1_p": _pack_dr(np.asarray(w_fc1, np.float32) * W1_S).astype(FP8),
        "w2_p": _pack_dr(np.asarray(w_fc2, np.float32) * W2_S).astype(FP8),
        "bp_s": np.asarray(b_proj, np.float32) * 512.0,
        "bf1_d": np.asarray(b_fc1, np.float32),
        "bf2_s": np.asarray(b_fc2, np.float32) * W2_S,
    }
    if ln1_affine:
        shared["ln1_g"] = ln1_g
        shared["ln1_b"] = ln1_b
    if ln2_affine:
        shared["ln2_g"] = ln2_g
        shared["ln2_b"] = ln2_b

    key = (ln1_affine, ln2_affine)
    if _NC_CACHE is None or _NC_KEY != key:
        _NC_CACHE = _build_bass(ln1_affine, ln2_affine)
        _NC_KEY = key

    in_maps = []
    for c in range(8):
        b, hh = c // 2, c % 2
        xbv = np.ascontiguousarray(np.roll(x[b], -hh * NO, axis=0))
        in_maps.append({"xb": xbv, **shared})

    res = run_bass_kernel_spmd(_NC_CACHE, in_maps, core_ids=list(range(8)))

    outp = np.empty((B, N, C), np.float32)
    for c in range(8):
        b, hh = c // 2, c % 2
        outp[b, hh * NO:(hh + 1) * NO, :] = res.results[c]["out"]
    return outp


# revision 17
# speedup vs baseline: 1.7294x; 1.0317x over previous
"""Trainium2 Bass kernel for a dense transformer block (B=4, N=2048, C=768, H=12).

Sharding: 8 cores = 4 batches x 2 sequence halves; each core's batch rows are
rolled so its own 1024 query rows are rows 0:1023 (softmax is permutation-
invariant over keys). Each core computes LN1 over all 2048 rows, full K/V,
attention + MLP for its own 1024 rows. No collectives.

All GEMMs run in fp8 (e4m3) DoubleRow perf mode: operands are [128, 2, *]
access patterns holding two stacked 128-deep contraction tiles (contraction
index c = p + 128*j + 256*instr), which the cost model streams at 0.5
cycles/output-column (4x the f32r rate). Weights are quantized+packed host-
side with power-of-two scales chosen to center fp8's dynamic range; descales
fold into activation scale arguments and output tensor_scalar copies.
Softmax: exp on ACT writes fp8 probability tiles that feed the attn@V
DoubleRow matmul directly; a 16-valued ones-column appended to V accumulates
the denominator on the same partition as its query row, so normalization is a
per-partition reciprocal+scale. Transposes run in bf16 (PE, 1 cycle/row).
"""

import numpy as np
import ml_dtypes

B, N, C = 4, 2048, 768
H, DH = 12, 64
HID = 4 * C
SCALE = DH ** -0.5
EPS = 1e-5

P = 128
NO = 1024           # own query rows per core
NT = N // P         # 16 token tiles
NOT_ = NO // P      # 8 own token tiles

FP8 = ml_dtypes.float8_e4m3
BF16NP = ml_dtypes.bfloat16

WQ_S = 16.0 * SCALE ** 0.5   # on w_q and w_k (so scores psum = 256*SCALE*qk)
WV_S = 16.0                  # on w_v; ones-column is 16 so the scale cancels
WP_S = 16.0
Y_S = 32.0                   # on normalized attention output
W1_S = 16.0
W2_S = 32.0


def _build_bass(ln1_affine, ln2_affine):
    import concourse.bass as bass
    import concourse.tile as tile
    from concourse import bacc, mybir
    from concourse.masks import make_identity
    from concourse.alu_op_type import AluOpType as A

    F32 = mybir.dt.float32
    F8 = mybir.dt.float8e4
    B16 = mybir.dt.bfloat16
    AF = mybir.ActivationFunctionType
    PM = mybir.MatmulPerfMode
    DR = PM.DoubleRow

    nc = bacc.Bacc("TRN2", target_bir_lowering=False, num_swdge_queues=4)

    xb = nc.dram_tensor("xb", [N, C], F32, kind="ExternalInput")
    wq_p = nc.dram_tensor("wq_p", [P, 3, 2, C], F8, kind="ExternalInput")
    wk_p = nc.dram_tensor("wk_p", [P, 3, 2, C], F8, kind="ExternalInput")
    wv_p = nc.dram_tensor("wv_p", [P, 3, 2, C], F8, kind="ExternalInput")
    wp_p = nc.dram_tensor("wp_p", [P, 3, 2, C], F8, kind="ExternalInput")
    w1_p = nc.dram_tensor("w1_p", [P, 3, 2, HID], F8, kind="ExternalInput")
    w2_p = nc.dram_tensor("w2_p", [P, 12, 2, C], F8, kind="ExternalInput")
    bp_s = nc.dram_tensor("bp_s", [C], F32, kind="ExternalInput")    # 512*b_proj
    bf1_d = nc.dram_tensor("bf1_d", [HID], F32, kind="ExternalInput")
    bf2_s = nc.dram_tensor("bf2_s", [C], F32, kind="ExternalInput")  # 32*b_fc2
    if ln1_affine:
        ln1_g = nc.dram_tensor("ln1_g", [C], F32, kind="ExternalInput")
        ln1_b = nc.dram_tensor("ln1_b", [C], F32, kind="ExternalInput")
    if ln2_affine:
        ln2_g = nc.dram_tensor("ln2_g", [C], F32, kind="ExternalInput")
        ln2_b = nc.dram_tensor("ln2_b", [C], F32, kind="ExternalInput")
    out_d = nc.dram_tensor("out", [NO, C], F32, kind="ExternalOutput")

    dma = nc.gpsimd.dma_start

    with tile.TileContext(nc) as tc:
        consts = tc.alloc_tile_pool(name="consts", bufs=1)
        pers = tc.alloc_tile_pool(name="pers", bufs=1)
        work = tc.alloc_tile_pool(name="work", bufs=2)
        attn = tc.alloc_tile_pool(name="attn", bufs=1)

        identf = consts.tile([P, P], F32)
        make_identity(nc, identf)
        identb = consts.tile([P, P], B16)
        nc.vector.tensor_copy(identb, identf)
        eps_t = consts.tile([P, 1], F32)
        nc.vector.memset(eps_t, EPS)
        bpT = consts.tile([P, 6], F32)
        bf1T = consts.tile([P, 24], F32)
        bf2T = consts.tile([P, 6], F32)

        # ---- persistent tiles
        x2 = pers.tile([P, NOT_, C], F32)
        h2T = pers.tile([P, 6, NO], F8)
        yT = pers.tile([P, 3, 2, NO], F8)
        mvs = pers.tile([P, NT, 2], F32)
        rsig = pers.tile([P, NT], F32)
        mvs2 = pers.tile([P, NOT_, 2], F32)
        rsig2 = pers.tile([P, NOT_], F32)
        w1_t = pers.tile([P, 3, 2, HID], F8)
        w2_t = pers.tile([P, 12, 2, C], F8)

        # ---- attention-phase tiles
        hT = attn.tile([P, 6, N], F8, tag="hT")
        QT = [attn.tile([96, 2, NO], F8, name="QT%d" % g) for g in range(4)]
        KT = [attn.tile([96, 2, N], F8, name="KT%d" % g) for g in range(4)]
        Vp = attn.tile([P, NT // 2, 2, H, 65], F8)
        y_sb = [attn.tile([P, NOT_, DH], B16, name="ysb%d" % h) for h in range(H)]
        wq_t = attn.tile([P, 3, 2, C], F8)
        wk_t = attn.tile([P, 3, 2, C], F8)
        wv_t = attn.tile([P, 3, 2, C], F8)
        wp_t = attn.tile([P, 3, 2, C], F8)

        if ln1_affine:
            g1_bc = attn.tile([P, C], F32)
            dma(out=g1_bc, in_=ln1_g[:].partition_broadcast(P))
            b1_bc = attn.tile([P, C], F32)
            dma(out=b1_bc, in_=ln1_b[:].partition_broadcast(P))
        if ln2_affine:
            g2_bc = pers.tile([P, C], F32)
            dma(out=g2_bc, in_=ln2_g[:].partition_broadcast(P))
            b2_bc = pers.tile([P, C], F32)
            dma(out=b2_bc, in_=ln2_b[:].partition_broadcast(P))

        nc.vector.memset(Vp[:, :, :, :, 64:65], 16.0)

        def ln_stats(xt, mvt, i):
            st = work.tile([P, 3, 6], F32, tag="st")
            for s in range(3):
                nc.vector.bn_stats(out=st[:, s, :], in_=xt[:, s * 256:(s + 1) * 256])
            nc.vector.bn_aggr(out=mvt[:, i, :], in_=st)

        def ln_rsqrt(mvt, rst, sl):
            lnv = work.tile([P, 8], F32, tag="lnv")
            n = sl.stop - sl.start
            nc.scalar.activation(out=lnv[:, 0:n], in_=mvt[:, sl, 1], func=AF.Ln,
                                 bias=eps_t)
            nc.scalar.activation(out=rst[:, sl], in_=lnv[:, 0:n], func=AF.Exp,
                                 scale=-0.5)

        def ln_normalize(xt, mvt, rst, i, g_bc, b_bc):
            # on GPSIMD: frees DVE, all-SBUF operands
            hb = work.tile([P, C], B16, tag="hb", bufs=2)
            with nc.allow_low_precision(reason="bf16 ln out"):
                nc.gpsimd.tensor_scalar(out=hb, in0=xt,
                                        scalar1=mvt[:, i, 0:1],
                                        scalar2=rst[:, i:i + 1],
                                        op0=A.subtract, op1=A.mult)
                if g_bc is not None:
                    nc.gpsimd.tensor_tensor(out=hb, in0=hb, in1=g_bc, op=A.mult)
                if b_bc is not None:
                    nc.gpsimd.tensor_tensor(out=hb, in0=hb, in1=b_bc, op=A.add)
            return hb

        # =========== Phase A: LN1 (+transpose to fp8 hT) and QKV projections
        def load_x_chunk(c0, ntile, tag="xf"):
            xt = work.tile([P, ntile, C], F32, tag=tag, bufs=4)
            dma(out=xt, in_=xb[c0 * P:(c0 + ntile) * P, :]
                .rearrange("(i p) c -> p i c", p=P))
            return xt

        with tc.tile_pool(name="ps_a", bufs=2, space="PSUM") as ps_a, \
             tc.tile_pool(name="ps_b", bufs=2, space="PSUM") as ps_b:

            def phase_a_half(half):
                # stats for all 8 tiles first, one batched rsqrt (2 act-table
                # loads per half instead of per-pair thrashing), then
                # normalize (gpsimd) + transpose + fp8 pack copies (DVE)
                xts = []
                for ch in range(4):
                    xt = load_x_chunk(8 * half + 2 * ch, 2)
                    xts.append(xt)
                    for k in range(2):
                        i = 8 * half + 2 * ch + k
                        ln_stats(xt[:, k, :], mvs, i)
                ln_rsqrt(mvs, rsig, slice(8 * half, 8 * half + 8))
                for ch in range(4):
                    for k in range(2):
                        i = 8 * half + 2 * ch + k
                        hb = ln_normalize(xts[ch][:, k, :], mvs, rsig, i,
                                          g1_bc if ln1_affine else None,
                                          b1_bc if ln1_affine else None)
                        tp = ps_a.tile([P, C], B16, tag="tr")
                        for t in range(6):
                            nc.tensor.transpose(tp[:, t * P:(t + 1) * P],
                                                hb[:, t * P:(t + 1) * P], identb)
                        with nc.allow_low_precision(reason="fp8 hT"):
                            nc.vector.tensor_copy(
                                out=hT[:, :, i * P:(i + 1) * P],
                                in_=tp[:].rearrange("p (t n) -> p t n", t=6))

            def emit_k(ch4):
                for g in range(4):
                    for dj in range(2):
                        cw = 96 * (2 * g + dj)
                        kp = ps_b.tile([P, 512], F32, tag="qk")
                        for ci in range(3):
                            nc.tensor.matmul(
                                kp[0:96, :], wk_t[:, ci, :, cw:cw + 96],
                                hT[:, 2 * ci:2 * ci + 2, ch4 * 512:(ch4 + 1) * 512],
                                start=(ci == 0), stop=(ci == 2), perf_mode=DR)
                        with nc.allow_low_precision(reason="fp8 kt"):
                            nc.vector.tensor_copy(
                                out=KT[g][:, dj, ch4 * 512:(ch4 + 1) * 512],
                                in_=kp[0:96, :])

            dma(out=wq_t, in_=wq_p[:, :, :, :])
            dma(out=wk_t, in_=wk_p[:, :, :, :])
            phase_a_half(0)
            dma(out=wv_t, in_=wv_p[:, :, :, :])
            dma(out=wp_t, in_=wp_p[:, :, :, :])

            # Q for own 1024 rows (PE waits only on hT[:, :, :1024])
            for g in range(4):
                for dj in range(2):
                    cw = 96 * (2 * g + dj)
                    for ch2 in range(2):
                        qp = ps_b.tile([P, 512], F32, tag="qk")
                        for ci in range(3):
                            nc.tensor.matmul(
                                qp[0:96, :], wq_t[:, ci, :, cw:cw + 96],
                                hT[:, 2 * ci:2 * ci + 2, ch2 * 512:(ch2 + 1) * 512],
                                start=(ci == 0), stop=(ci == 2), perf_mode=DR)
                        with nc.allow_low_precision(reason="fp8 qt"):
                            nc.vector.tensor_copy(
                                out=QT[g][:, dj, ch2 * 512:(ch2 + 1) * 512],
                                in_=qp[0:96, :])
            emit_k(0)
            emit_k(1)

            phase_a_half(1)
            dma(out=bpT, in_=bp_s[:].rearrange("(t p) -> p t", p=P))
            dma(out=bf1T, in_=bf1_d[:].rearrange("(t p) -> p t", p=P))
            dma(out=bf2T, in_=bf2_s[:].rearrange("(t p) -> p t", p=P))
            emit_k(2)
            emit_k(3)

            # V over all rows, [token, vdim] tiles -> paired Vp layout
            for i in range(NT):
                for pg in range(3):
                    vp = ps_b.tile([P, 256], F32, tag="v")
                    for ci in range(3):
                        nc.tensor.matmul(
                            vp, hT[:, 2 * ci:2 * ci + 2, i * P:(i + 1) * P],
                            wv_t[:, ci, :, 256 * pg:256 * (pg + 1)],
                            start=(ci == 0), stop=(ci == 2), perf_mode=DR)
                    with nc.allow_low_precision(reason="fp8 v"):
                        nc.vector.tensor_copy(
                            out=Vp[:, i // 2, i % 2, 4 * pg:4 * pg + 4, 0:64],
                            in_=vp[:].rearrange("p (h d) -> p h d", h=4))

        # MLP weights arrive during attention
        dma(out=w1_t, in_=w1_p[:, :, :, :])
        dma(out=w2_t, in_=w2_p[:, :, :, :])

        # =========== Phase C: attention per head
        with tc.tile_pool(name="ps_s", bufs=2, space="PSUM") as ps_s, \
             tc.tile_pool(name="ps_y", bufs=1, space="PSUM") as ps_y, \
             tc.tile_pool(name="ps_t", bufs=2, space="PSUM") as ps_t:
            for h in range(H):
                g, p0 = h // 3, 32 * (h % 3)
                ya = [ps_y.tile([P, 4, 65], F32, tag="ya%d" % half,
                                name="ya_%d_%d" % (h, half)) for half in range(2)]
                for mp in range(NT // 2):
                    eA = work.tile([P, 2, NO], F8, tag="eA")
                    for sub in range(2):
                        m = 2 * mp + sub
                        sp = ps_s.tile([P, NO], F32, tag="s")
                        for ch2 in range(2):
                            nc.tensor.matmul(
                                sp[:, ch2 * 512:(ch2 + 1) * 512],
                                KT[g][p0:p0 + 32, :, m * P:(m + 1) * P],
                                QT[g][p0:p0 + 32, :, ch2 * 512:(ch2 + 1) * 512],
                                start=True, stop=True, perf_mode=DR)
                        with nc.allow_low_precision(reason="fp8 probs"):
                            nc.scalar.activation(out=eA[:, sub, :], in_=sp,
                                                 func=AF.Exp, scale=1.0 / 256.0)
                    for qt in range(NOT_):
                        nc.tensor.matmul(
                            ya[qt // 4][:, qt % 4, :],
                            eA[:, :, qt * P:(qt + 1) * P],
                            Vp[:, mp, :, h, :],
                            start=(mp == 0), stop=(mp == NT // 2 - 1),
                            perf_mode=DR)
                rin = work.tile([P, NOT_], F32, tag="rin")
                with nc.allow_low_precision(reason="softmax denom recip"):
                    nc.vector.reciprocal(rin[:, 0:4], ya[0][:, :, 64])
                    nc.vector.reciprocal(rin[:, 4:8], ya[1][:, :, 64])
                for qt in range(NOT_):
                    with nc.allow_low_precision(reason="bf16 y"):
                        nc.vector.tensor_scalar(
                            out=y_sb[h][:, qt, :],
                            in0=ya[qt // 4][:, qt % 4, 0:64],
                            scalar1=rin[:, qt:qt + 1], scalar2=Y_S,
                            op0=A.mult, op1=A.mult)
                # transpose pair of heads into yT once the odd head lands
                if h % 2 == 1:
                    i3, jj = h // 4, (h % 4) // 2
                    for qt in range(NOT_):
                        ytp = ps_t.tile([P, P], B16, tag="yt")
                        nc.tensor.transpose(ytp[0:64, :], y_sb[h - 1][:, qt, :],
                                            identb)
                        nc.tensor.transpose(ytp[64:128, :], y_sb[h][:, qt, :],
                                            identb)
                        with nc.allow_low_precision(reason="fp8 yT"):
                            nc.vector.tensor_copy(
                                out=yT[:, i3, jj, qt * P:(qt + 1) * P], in_=ytp)

        # =========== Phase D: output projection
        # attnT reuses hT's 12KB slot (hT last read by the V matmuls)
        attnT = attn.tile([P, 6, NO], B16, tag="hT")
        with tc.tile_pool(name="ps_p", bufs=2, space="PSUM") as ps_p:
            for jt in range(6):
                for ch2 in range(2):
                    pp = ps_p.tile([P, 512], F32, tag="pp")
                    for ci in range(3):
                        nc.tensor.matmul(
                            pp, wp_t[:, ci, :, jt * P:(jt + 1) * P],
                            yT[:, ci, :, ch2 * 512:(ch2 + 1) * 512],
                            start=(ci == 0), stop=(ci == 2), perf_mode=DR)
                    with nc.allow_low_precision(reason="bf16 attnT"):
                        nc.vector.tensor_scalar(
                            out=attnT[:, jt, ch2 * 512:(ch2 + 1) * 512], in0=pp,
                            scalar1=bpT[:, jt:jt + 1], scalar2=1.0 / 512.0,
                            op0=A.add, op1=A.mult)

        # =========== Phase E: residual + LN2 -> h2T
        with tc.tile_pool(name="ps_e", bufs=3, space="PSUM") as ps_e:
            for i in range(NOT_):
                if i % 2 == 0:
                    xo = load_x_chunk(i, 2, tag="xf")
                tr = ps_e.tile([P, C], B16, tag="tr")
                for t in range(6):
                    nc.tensor.transpose(tr[:, t * P:(t + 1) * P],
                                        attnT[:, t, i * P:(i + 1) * P], identb)
                nc.vector.tensor_tensor(out=x2[:, i, :], in0=tr,
                                        in1=xo[:, i % 2, :], op=A.add)
                ln_stats(x2[:, i, :], mvs2, i)
            ln_rsqrt(mvs2, rsig2, slice(0, NOT_))
            for i in range(NOT_):
                hb2 = ln_normalize(x2[:, i, :], mvs2, rsig2, i,
                                   g2_bc if ln2_affine else None,
                                   b2_bc if ln2_affine else None)
                tp2 = ps_e.tile([P, C], B16, tag="tr")
                for t in range(6):
                    nc.tensor.transpose(tp2[:, t * P:(t + 1) * P],
                                        hb2[:, t * P:(t + 1) * P], identb)
                with nc.allow_low_precision(reason="fp8 h2T"):
                    nc.vector.tensor_copy(
                        out=h2T[:, :, i * P:(i + 1) * P],
                        in_=tp2[:].rearrange("p (t n) -> p t n", t=6))

        attn.release()

        # =========== Phase F: MLP per 512-token half + output
        for nh in range(2):
            sl = slice(nh * 512, (nh + 1) * 512)
            with tc.tile_pool(name="ps_m%d" % nh, bufs=1, space="PSUM") as ps_m:
                f2s = [ps_m.tile([P, 512], F32, tag="f2c%d" % jt,
                                 name="f2acc%d_%d" % (nh, jt))
                       for jt in range(6)]
                def emit_fc1(t2):
                    gh = work.tile([P, 2, 512], F8, tag="ghat", bufs=3)
                    for j2 in range(2):
                        fps = ps_m.tile([P, 512], F32, tag="f1", bufs=2)
                        hw = P * (2 * t2 + j2)
                        for ci in range(3):
                            nc.tensor.matmul(
                                fps, w1_t[:, ci, :, hw:hw + P],
                                h2T[:, 2 * ci:2 * ci + 2, sl],
                                start=(ci == 0), stop=(ci == 2), perf_mode=DR)
                        with nc.allow_low_precision(reason="fp8 gelu"):
                            nc.scalar.activation(
                                out=gh[:, j2, :], in_=fps, func=AF.Gelu,
                                scale=1.0 / W1_S,
                                bias=bf1T[:, 2 * t2 + j2:2 * t2 + j2 + 1])
                    return gh

                # software pipeline: fc1(t2+1) is emitted before fc2(t2) so
                # the PE never stalls waiting on gelu
                gh_cur = emit_fc1(0)
                for t2 in range(12):
                    gh_next = emit_fc1(t2 + 1) if t2 < 11 else None
                    for jt in range(6):
                        nc.tensor.matmul(
                            f2s[jt], w2_t[:, t2, :, jt * P:(jt + 1) * P],
                            gh_cur,
                            start=(t2 == 0), stop=(t2 == 11), perf_mode=DR)
                    gh_cur = gh_next
                mlpT = work.tile([P, 6, 512], B16, tag="mlpT")
                for jt in range(6):
                    with nc.allow_low_precision(reason="bf16 mlpT"):
                        nc.vector.tensor_scalar(
                            out=mlpT[:, jt, :], in0=f2s[jt],
                            scalar1=bf2T[:, jt:jt + 1], scalar2=1.0 / W2_S,
                            op0=A.add, op1=A.mult)
            with tc.tile_pool(name="ps_o%d" % nh, bufs=2, space="PSUM") as ps_o:
                for it in range(4):
                    i = 4 * nh + it
                    tro = ps_o.tile([P, C], B16, tag="tro")
                    for t in range(6):
                        nc.tensor.transpose(tro[:, t * P:(t + 1) * P],
                                            mlpT[:, t, it * P:(it + 1) * P],
                                            identb)
                    o_sb = work.tile([P, C], F32, tag="osb")
                    nc.vector.tensor_tensor(out=o_sb, in0=tro, in1=x2[:, i, :],
                                            op=A.add)
                    dma(out=out_d[i * P:(i + 1) * P, :], in_=o_sb)

        work.release()
        pers.release()
        consts.release()

    nc.compile()
    return nc


def _pack_dr(w):
    """[C_contract, cols] -> [128, C/256, 2, cols] with c = p + 128*j + 256*i."""
    ctr, cols = w.shape
    return np.ascontiguousarray(
        w.reshape(ctr // 256, 2, 128, cols).transpose(2, 0, 1, 3))


def _reorder_qk(w):
    """Reorder head-dim cols: (tg, h3, dj, p32) -> (tg, dj, h3, p32)."""
    return np.ascontiguousarray(
        w.reshape(C, 4, 3, 2, 32).transpose(0, 1, 3, 2, 4).reshape(C, C))


_NC_CACHE = None
_NC_KEY = None


def kernel(x, ln1_g, ln1_b, w_qkv, w_proj, b_proj, ln2_g, ln2_b,
           w_fc1, b_fc1, w_fc2, b_fc2):
    global _NC_CACHE, _NC_KEY
    from concourse.bass_utils import run_bass_kernel_spmd

    x = np.asarray(x, np.float32)
    w_qkv = np.asarray(w_qkv, np.float32)
    ln1_g = np.asarray(ln1_g, np.float32)
    ln1_b = np.asarray(ln1_b, np.float32)
    ln2_g = np.asarray(ln2_g, np.float32)
    ln2_b = np.asarray(ln2_b, np.float32)

    ln1_affine = not (np.all(ln1_g == 1.0) and np.all(ln1_b == 0.0))
    ln2_affine = not (np.all(ln2_g == 1.0) and np.all(ln2_b == 0.0))

    wq = _reorder_qk(w_qkv[:, 0:C]) * WQ_S
    wk = _reorder_qk(w_qkv[:, C:2 * C]) * WQ_S
    wv = w_qkv[:, 2 * C:3 * C] * WV_S

    shared = {
        "wq_p": _pack_dr(wq).astype(FP8),
        "wk_p": _pack_dr(wk).astype(FP8),
        "wv_p": _pack_dr(wv).astype(FP8),
        "wp_p": _pack_dr(np.asarray(w_proj, np.float32) * WP_S).astype(FP8),
        "w1_p": _pack_dr(np.asarray(w_fc1, np.float32) * W1_S).astype(FP8),
        "w2_p": _pack_dr(np.asarray(w_fc2, np.float32) * W2_S).astype(FP8),
        "bp_s": np.asarray(b_proj, np.float32) * 512.0,
        "bf1_d": np.asarray(b_fc1, np.float32),
        "bf2_s": np.asarray(b_fc2, np.float32) * W2_S,
    }
    if ln1_affine:
        shared["ln1_g"] = ln1_g
        shared["ln1_b"] = ln1_b
    if ln2_affine:
        shared["ln2_g"] = ln2_g
        shared["ln2_b"] = ln2_b

    key = (ln1_affine, ln2_affine)
    if _NC_CACHE is None or _NC_KEY != key:
        _NC_CACHE = _build_bass(ln1_affine, ln2_affine)
        _NC_KEY = key

    in_maps = []
    for c in range(8):
        b, hh = c // 2, c % 2
        xbv = np.ascontiguousarray(np.roll(x[b], -hh * NO, axis=0))
        in_maps.append({"xb": xbv, **shared})

    res = run_bass_kernel_spmd(_NC_CACHE, in_maps, core_ids=list(range(8)))

    outp = np.empty((B, N, C), np.float32)
    for c in range(8):
        b, hh = c // 2, c % 2
        outp[b, hh * NO:(hh + 1) * NO, :] = res.results[c]["out"]
    return outp


# revision 18
# speedup vs baseline: 1.7313x; 1.0011x over previous
"""Trainium2 Bass kernel for a dense transformer block (B=4, N=2048, C=768, H=12).

Sharding: 8 cores = 4 batches x 2 sequence halves; each core's batch rows are
rolled so its own 1024 query rows are rows 0:1023 (softmax is permutation-
invariant over keys). Each core computes LN1 over all 2048 rows, full K/V,
attention + MLP for its own 1024 rows. No collectives.

All GEMMs run in fp8 (e4m3) DoubleRow perf mode: operands are [128, 2, *]
access patterns holding two stacked 128-deep contraction tiles (contraction
index c = p + 128*j + 256*instr), which the cost model streams at 0.5
cycles/output-column (4x the f32r rate). Weights are quantized+packed host-
side with power-of-two scales chosen to center fp8's dynamic range; descales
fold into activation scale arguments and output tensor_scalar copies.
Softmax: exp on ACT writes fp8 probability tiles that feed the attn@V
DoubleRow matmul directly; a 16-valued ones-column appended to V accumulates
the denominator on the same partition as its query row, so normalization is a
per-partition reciprocal+scale. Transposes run in bf16 (PE, 1 cycle/row).
"""

import numpy as np
import ml_dtypes

B, N, C = 4, 2048, 768
H, DH = 12, 64
HID = 4 * C
SCALE = DH ** -0.5
EPS = 1e-5

P = 128
NO = 1024           # own query rows per core
NT = N // P         # 16 token tiles
NOT_ = NO // P      # 8 own token tiles

FP8 = ml_dtypes.float8_e4m3
BF16NP = ml_dtypes.bfloat16

WQ_S = 16.0 * SCALE ** 0.5   # on w_q and w_k (so scores psum = 256*SCALE*qk)
WV_S = 16.0                  # on w_v; ones-column is 16 so the scale cancels
WP_S = 16.0
Y_S = 32.0                   # on normalized attention output
W1_S = 16.0
W2_S = 32.0


def _build_bass(ln1_affine, ln2_affine):
    import concourse.bass as bass
    import concourse.tile as tile
    from concourse import bacc, mybir
    from concourse.masks import make_identity
    from concourse.alu_op_type import AluOpType as A

    F32 = mybir.dt.float32
    F8 = mybir.dt.float8e4
    B16 = mybir.dt.bfloat16
    AF = mybir.ActivationFunctionType
    PM = mybir.MatmulPerfMode
    DR = PM.DoubleRow

    nc = bacc.Bacc("TRN2", target_bir_lowering=False, num_swdge_queues=4)

    xb = nc.dram_tensor("xb", [N, C], F32, kind="ExternalInput")
    wq_p = nc.dram_tensor("wq_p", [P, 3, 2, C], F8, kind="ExternalInput")
    wk_p = nc.dram_tensor("wk_p", [P, 3, 2, C], F8, kind="ExternalInput")
    wv_p = nc.dram_tensor("wv_p", [P, 3, 2, C], F8, kind="ExternalInput")
    wp_p = nc.dram_tensor("wp_p", [P, 3, 2, C], F8, kind="ExternalInput")
    w1_p = nc.dram_tensor("w1_p", [P, 3, 2, HID], F8, kind="ExternalInput")
    w2_p = nc.dram_tensor("w2_p", [P, 12, 2, C], F8, kind="ExternalInput")
    bp_s = nc.dram_tensor("bp_s", [C], F32, kind="ExternalInput")    # 512*b_proj
    bf1_d = nc.dram_tensor("bf1_d", [HID], F32, kind="ExternalInput")
    bf2_s = nc.dram_tensor("bf2_s", [C], F32, kind="ExternalInput")  # 32*b_fc2
    if ln1_affine:
        ln1_g = nc.dram_tensor("ln1_g", [C], F32, kind="ExternalInput")
        ln1_b = nc.dram_tensor("ln1_b", [C], F32, kind="ExternalInput")
    if ln2_affine:
        ln2_g = nc.dram_tensor("ln2_g", [C], F32, kind="ExternalInput")
        ln2_b = nc.dram_tensor("ln2_b", [C], F32, kind="ExternalInput")
    out_d = nc.dram_tensor("out", [NO, C], F32, kind="ExternalOutput")

    dma = nc.sync.dma_start

    with tile.TileContext(nc) as tc:
        consts = tc.alloc_tile_pool(name="consts", bufs=1)
        pers = tc.alloc_tile_pool(name="pers", bufs=1)
        work = tc.alloc_tile_pool(name="work", bufs=2)
        attn = tc.alloc_tile_pool(name="attn", bufs=1)

        identf = consts.tile([P, P], F32)
        make_identity(nc, identf)
        identb = consts.tile([P, P], B16)
        nc.vector.tensor_copy(identb, identf)
        eps_t = consts.tile([P, 1], F32)
        nc.vector.memset(eps_t, EPS)
        bpT = consts.tile([P, 6], F32)
        bf1T = consts.tile([P, 24], F32)
        bf2T = consts.tile([P, 6], F32)

        # ---- persistent tiles
        x2 = pers.tile([P, NOT_, C], F32)
        h2T = pers.tile([P, 6, NO], F8)
        yT = pers.tile([P, 3, 2, NO], F8)
        mvs = pers.tile([P, NT, 2], F32)
        rsig = pers.tile([P, NT], F32)
        mvs2 = pers.tile([P, NOT_, 2], F32)
        rsig2 = pers.tile([P, NOT_], F32)
        w1_t = pers.tile([P, 3, 2, HID], F8)
        w2_t = pers.tile([P, 12, 2, C], F8)

        # ---- attention-phase tiles
        hT = attn.tile([P, 6, N], F8, tag="hT")
        QT = [attn.tile([96, 2, NO], F8, name="QT%d" % g) for g in range(4)]
        KT = [attn.tile([96, 2, N], F8, name="KT%d" % g) for g in range(4)]
        Vp = attn.tile([P, NT // 2, 2, H, 65], F8)
        y_sb = [attn.tile([P, NOT_, DH], B16, name="ysb%d" % h) for h in range(H)]
        wq_t = attn.tile([P, 3, 2, C], F8)
        wk_t = attn.tile([P, 3, 2, C], F8)
        wv_t = attn.tile([P, 3, 2, C], F8)
        wp_t = attn.tile([P, 3, 2, C], F8)

        if ln1_affine:
            g1_bc = attn.tile([P, C], F32)
            dma(out=g1_bc, in_=ln1_g[:].partition_broadcast(P))
            b1_bc = attn.tile([P, C], F32)
            dma(out=b1_bc, in_=ln1_b[:].partition_broadcast(P))
        if ln2_affine:
            g2_bc = pers.tile([P, C], F32)
            dma(out=g2_bc, in_=ln2_g[:].partition_broadcast(P))
            b2_bc = pers.tile([P, C], F32)
            dma(out=b2_bc, in_=ln2_b[:].partition_broadcast(P))

        nc.vector.memset(Vp[:, :, :, :, 64:65], 16.0)

        def ln_stats(xt, mvt, i):
            st = work.tile([P, 3, 6], F32, tag="st")
            for s in range(3):
                nc.vector.bn_stats(out=st[:, s, :], in_=xt[:, s * 256:(s + 1) * 256])
            nc.vector.bn_aggr(out=mvt[:, i, :], in_=st)

        def ln_rsqrt(mvt, rst, sl):
            lnv = work.tile([P, 8], F32, tag="lnv")
            n = sl.stop - sl.start
            nc.scalar.activation(out=lnv[:, 0:n], in_=mvt[:, sl, 1], func=AF.Ln,
                                 bias=eps_t)
            nc.scalar.activation(out=rst[:, sl], in_=lnv[:, 0:n], func=AF.Exp,
                                 scale=-0.5)

        def ln_normalize(xt, mvt, rst, i, g_bc, b_bc):
            # on GPSIMD: frees DVE, all-SBUF operands
            hb = work.tile([P, C], B16, tag="hb", bufs=2)
            with nc.allow_low_precision(reason="bf16 ln out"):
                nc.gpsimd.tensor_scalar(out=hb, in0=xt,
                                        scalar1=mvt[:, i, 0:1],
                                        scalar2=rst[:, i:i + 1],
                                        op0=A.subtract, op1=A.mult)
                if g_bc is not None:
                    nc.gpsimd.tensor_tensor(out=hb, in0=hb, in1=g_bc, op=A.mult)
                if b_bc is not None:
                    nc.gpsimd.tensor_tensor(out=hb, in0=hb, in1=b_bc, op=A.add)
            return hb

        # =========== Phase A: LN1 (+transpose to fp8 hT) and QKV projections
        def load_x_chunk(c0, ntile, tag="xf"):
            xt = work.tile([P, ntile, C], F32, tag=tag, bufs=4)
            dma(out=xt, in_=xb[c0 * P:(c0 + ntile) * P, :]
                .rearrange("(i p) c -> p i c", p=P))
            return xt

        with tc.tile_pool(name="ps_a", bufs=2, space="PSUM") as ps_a, \
             tc.tile_pool(name="ps_b", bufs=2, space="PSUM") as ps_b:

            def phase_a_half(half):
                # stats for all 8 tiles first, one batched rsqrt (2 act-table
                # loads per half instead of per-pair thrashing), then
                # normalize (gpsimd) + transpose + fp8 pack copies (DVE)
                xts = []
                for ch in range(4):
                    xt = load_x_chunk(8 * half + 2 * ch, 2)
                    xts.append(xt)
                    for k in range(2):
                        i = 8 * half + 2 * ch + k
                        ln_stats(xt[:, k, :], mvs, i)
                ln_rsqrt(mvs, rsig, slice(8 * half, 8 * half + 8))
                for ch in range(4):
                    for k in range(2):
                        i = 8 * half + 2 * ch + k
                        hb = ln_normalize(xts[ch][:, k, :], mvs, rsig, i,
                                          g1_bc if ln1_affine else None,
                                          b1_bc if ln1_affine else None)
                        tp = ps_a.tile([P, C], B16, tag="tr")
                        for t in range(6):
                            nc.tensor.transpose(tp[:, t * P:(t + 1) * P],
                                                hb[:, t * P:(t + 1) * P], identb)
                        with nc.allow_low_precision(reason="fp8 hT"):
                            nc.vector.tensor_copy(
                                out=hT[:, :, i * P:(i + 1) * P],
                                in_=tp[:].rearrange("p (t n) -> p t n", t=6))

            def emit_k(ch4):
                for g in range(4):
                    for dj in range(2):
                        cw = 96 * (2 * g + dj)
                        kp = ps_b.tile([P, 512], F32, tag="qk")
                        for ci in range(3):
                            nc.tensor.matmul(
                                kp[0:96, :], wk_t[:, ci, :, cw:cw + 96],
                                hT[:, 2 * ci:2 * ci + 2, ch4 * 512:(ch4 + 1) * 512],
                                start=(ci == 0), stop=(ci == 2), perf_mode=DR)
                        with nc.allow_low_precision(reason="fp8 kt"):
                            nc.vector.tensor_copy(
                                out=KT[g][:, dj, ch4 * 512:(ch4 + 1) * 512],
                                in_=kp[0:96, :])

            dma(out=wq_t, in_=wq_p[:, :, :, :])
            dma(out=wk_t, in_=wk_p[:, :, :, :])
            phase_a_half(0)
            dma(out=wv_t, in_=wv_p[:, :, :, :])
            dma(out=wp_t, in_=wp_p[:, :, :, :])

            # Q for own 1024 rows (PE waits only on hT[:, :, :1024])
            for g in range(4):
                for dj in range(2):
                    cw = 96 * (2 * g + dj)
                    for ch2 in range(2):
                        qp = ps_b.tile([P, 512], F32, tag="qk")
                        for ci in range(3):
                            nc.tensor.matmul(
                                qp[0:96, :], wq_t[:, ci, :, cw:cw + 96],
                                hT[:, 2 * ci:2 * ci + 2, ch2 * 512:(ch2 + 1) * 512],
                                start=(ci == 0), stop=(ci == 2), perf_mode=DR)
                        with nc.allow_low_precision(reason="fp8 qt"):
                            nc.vector.tensor_copy(
                                out=QT[g][:, dj, ch2 * 512:(ch2 + 1) * 512],
                                in_=qp[0:96, :])
            emit_k(0)
            emit_k(1)

            def emit_v(i0, i1):
                # V [token, vdim] tiles -> paired Vp layout
                for i in range(i0, i1):
                    for pg in range(3):
                        vp = ps_b.tile([P, 256], F32, tag="v")
                        for ci in range(3):
                            nc.tensor.matmul(
                                vp, hT[:, 2 * ci:2 * ci + 2, i * P:(i + 1) * P],
                                wv_t[:, ci, :, 256 * pg:256 * (pg + 1)],
                                start=(ci == 0), stop=(ci == 2), perf_mode=DR)
                        with nc.allow_low_precision(reason="fp8 v"):
                            nc.vector.tensor_copy(
                                out=Vp[:, i // 2, i % 2, 4 * pg:4 * pg + 4, 0:64],
                                in_=vp[:].rearrange("p (h d) -> p h d", h=4))

            emit_v(0, 8)
            phase_a_half(1)
            dma(out=bpT, in_=bp_s[:].rearrange("(t p) -> p t", p=P))
            dma(out=bf1T, in_=bf1_d[:].rearrange("(t p) -> p t", p=P))
            dma(out=bf2T, in_=bf2_s[:].rearrange("(t p) -> p t", p=P))
            emit_k(2)
            emit_k(3)
            emit_v(8, NT)

        # MLP weights arrive during attention
        dma(out=w1_t, in_=w1_p[:, :, :, :])
        dma(out=w2_t, in_=w2_p[:, :, :, :])

        # =========== Phase C: attention per head
        with tc.tile_pool(name="ps_s", bufs=2, space="PSUM") as ps_s, \
             tc.tile_pool(name="ps_y", bufs=1, space="PSUM") as ps_y, \
             tc.tile_pool(name="ps_t", bufs=2, space="PSUM") as ps_t:
            for h in range(H):
                g, p0 = h // 3, 32 * (h % 3)
                ya = [ps_y.tile([P, 4, 65], F32, tag="ya%d" % half,
                                name="ya_%d_%d" % (h, half)) for half in range(2)]
                for mp in range(NT // 2):
                    eA = work.tile([P, 2, NO], F8, tag="eA", bufs=4)
                    for sub in range(2):
                        m = 2 * mp + sub
                        sp = ps_s.tile([P, NO], F32, tag="s")
                        for ch2 in range(2):
                            nc.tensor.matmul(
                                sp[:, ch2 * 512:(ch2 + 1) * 512],
                                KT[g][p0:p0 + 32, :, m * P:(m + 1) * P],
                                QT[g][p0:p0 + 32, :, ch2 * 512:(ch2 + 1) * 512],
                                start=True, stop=True, perf_mode=DR)
                        with nc.allow_low_precision(reason="fp8 probs"):
                            nc.scalar.activation(out=eA[:, sub, :], in_=sp,
                                                 func=AF.Exp, scale=1.0 / 256.0)
                    for qt in range(NOT_):
                        nc.tensor.matmul(
                            ya[qt // 4][:, qt % 4, :],
                            eA[:, :, qt * P:(qt + 1) * P],
                            Vp[:, mp, :, h, :],
                            start=(mp == 0), stop=(mp == NT // 2 - 1),
                            perf_mode=DR)
                rin = work.tile([P, NOT_], F32, tag="rin")
                with nc.allow_low_precision(reason="softmax denom recip"):
                    nc.vector.reciprocal(rin[:, 0:4], ya[0][:, :, 64])
                    nc.vector.reciprocal(rin[:, 4:8], ya[1][:, :, 64])
                for qt in range(NOT_):
                    with nc.allow_low_precision(reason="bf16 y"):
                        nc.vector.tensor_scalar(
                            out=y_sb[h][:, qt, :],
                            in0=ya[qt // 4][:, qt % 4, 0:64],
                            scalar1=rin[:, qt:qt + 1], scalar2=Y_S,
                            op0=A.mult, op1=A.mult)
                # transpose pair of heads into yT once the odd head lands
                if h % 2 == 1:
                    i3, jj = h // 4, (h % 4) // 2
                    for qt in range(NOT_):
                        ytp = ps_t.tile([P, P], B16, tag="yt")
                        nc.tensor.transpose(ytp[0:64, :], y_sb[h - 1][:, qt, :],
                                            identb)
                        nc.tensor.transpose(ytp[64:128, :], y_sb[h][:, qt, :],
                                            identb)
                        with nc.allow_low_precision(reason="fp8 yT"):
                            nc.vector.tensor_copy(
                                out=yT[:, i3, jj, qt * P:(qt + 1) * P], in_=ytp)

        # =========== Phase D: output projection
        # attnT reuses hT's 12KB slot (hT last read by the V matmuls)
        attnT = attn.tile([P, 6, NO], B16, tag="hT")
        with tc.tile_pool(name="ps_p", bufs=2, space="PSUM") as ps_p:
            for jt in range(6):
                for ch2 in range(2):
                    pp = ps_p.tile([P, 512], F32, tag="pp")
                    for ci in range(3):
                        nc.tensor.matmul(
                            pp, wp_t[:, ci, :, jt * P:(jt + 1) * P],
                            yT[:, ci, :, ch2 * 512:(ch2 + 1) * 512],
                            start=(ci == 0), stop=(ci == 2), perf_mode=DR)
                    with nc.allow_low_precision(reason="bf16 attnT"):
                        nc.vector.tensor_scalar(
                            out=attnT[:, jt, ch2 * 512:(ch2 + 1) * 512], in0=pp,
                            scalar1=bpT[:, jt:jt + 1], scalar2=1.0 / 512.0,
                            op0=A.add, op1=A.mult)

        # =========== Phase E: residual + LN2 -> h2T
        with tc.tile_pool(name="ps_e", bufs=3, space="PSUM") as ps_e:
            for i in range(NOT_):
                if i % 2 == 0:
                    xo = load_x_chunk(i, 2, tag="xf")
                tr = ps_e.tile([P, C], B16, tag="tr")
                for t in range(6):
                    nc.tensor.transpose(tr[:, t * P:(t + 1) * P],
                                        attnT[:, t, i * P:(i + 1) * P], identb)
                nc.vector.tensor_tensor(out=x2[:, i, :], in0=tr,
                                        in1=xo[:, i % 2, :], op=A.add)
                ln_stats(x2[:, i, :], mvs2, i)
            ln_rsqrt(mvs2, rsig2, slice(0, NOT_))
            for i in range(NOT_):
                hb2 = ln_normalize(x2[:, i, :], mvs2, rsig2, i,
                                   g2_bc if ln2_affine else None,
                                   b2_bc if ln2_affine else None)
                tp2 = ps_e.tile([P, C], B16, tag="tr")
                for t in range(6):
                    nc.tensor.transpose(tp2[:, t * P:(t + 1) * P],
                                        hb2[:, t * P:(t + 1) * P], identb)
                with nc.allow_low_precision(reason="fp8 h2T"):
                    nc.vector.tensor_copy(
                        out=h2T[:, :, i * P:(i + 1) * P],
                        in_=tp2[:].rearrange("p (t n) -> p t n", t=6))

        attn.release()

        # =========== Phase F: MLP per 512-token half + output
        for nh in range(2):
            sl = slice(nh * 512, (nh + 1) * 512)
            with tc.tile_pool(name="ps_m%d" % nh, bufs=1, space="PSUM") as ps_m:
                f2s = [ps_m.tile([P, 512], F32, tag="f2c%d" % jt,
                                 name="f2acc%d_%d" % (nh, jt))
                       for jt in range(6)]
                def emit_fc1(t2):
                    gh = work.tile([P, 2, 512], F8, tag="ghat", bufs=3)
                    for j2 in range(2):
                        fps = ps_m.tile([P, 512], F32, tag="f1", bufs=2)
                        hw = P * (2 * t2 + j2)
                        for ci in range(3):
                            nc.tensor.matmul(
                                fps, w1_t[:, ci, :, hw:hw + P],
                                h2T[:, 2 * ci:2 * ci + 2, sl],
                                start=(ci == 0), stop=(ci == 2), perf_mode=DR)
                        with nc.allow_low_precision(reason="fp8 gelu"):
                            nc.scalar.activation(
                                out=gh[:, j2, :], in_=fps, func=AF.Gelu,
                                scale=1.0 / W1_S,
                                bias=bf1T[:, 2 * t2 + j2:2 * t2 + j2 + 1])
                    return gh

                # software pipeline: fc1(t2+1) is emitted before fc2(t2) so
                # the PE never stalls waiting on gelu
                gh_cur = emit_fc1(0)
                for t2 in range(12):
                    gh_next = emit_fc1(t2 + 1) if t2 < 11 else None
                    for jt in range(6):
                        nc.tensor.matmul(
                            f2s[jt], w2_t[:, t2, :, jt * P:(jt + 1) * P],
                            gh_cur,
                            start=(t2 == 0), stop=(t2 == 11), perf_mode=DR)
                    gh_cur = gh_next
                mlpT = work.tile([P, 6, 512], B16, tag="mlpT")
                for jt in range(6):
                    with nc.allow_low_precision(reason="bf16 mlpT"):
                        nc.vector.tensor_scalar(
                            out=mlpT[:, jt, :], in0=f2s[jt],
                            scalar1=bf2T[:, jt:jt + 1], scalar2=1.0 / W2_S,
                            op0=A.add, op1=A.mult)
            with tc.tile_pool(name="ps_o%d" % nh, bufs=2, space="PSUM") as ps_o:
                for it in range(4):
                    i = 4 * nh + it
                    tro = ps_o.tile([P, C], B16, tag="tro")
                    for t in range(6):
                        nc.tensor.transpose(tro[:, t * P:(t + 1) * P],
                                            mlpT[:, t, it * P:(it + 1) * P],
                                            identb)
                    o_sb = work.tile([P, C], F32, tag="osb")
                    nc.vector.tensor_tensor(out=o_sb, in0=tro, in1=x2[:, i, :],
                                            op=A.add)
                    dma(out=out_d[i * P:(i + 1) * P, :], in_=o_sb)

        work.release()
        pers.release()
        consts.release()

    nc.compile()
    return nc


def _pack_dr(w):
    """[C_contract, cols] -> [128, C/256, 2, cols] with c = p + 128*j + 256*i."""
    ctr, cols = w.shape
    return np.ascontiguousarray(
        w.reshape(ctr // 256, 2, 128, cols).transpose(2, 0, 1, 3))


def _reorder_qk(w):
    """Reorder head-dim cols: (tg, h3, dj, p32) -> (tg, dj, h3, p32)."""
    return np.ascontiguousarray(
        w.reshape(C, 4, 3, 2, 32).transpose(0, 1, 3, 2, 4).reshape(C, C))


_NC_CACHE = None
_NC_KEY = None


def kernel(x, ln1_g, ln1_b, w_qkv, w_proj, b_proj, ln2_g, ln2_b,
           w_fc1, b_fc1, w_fc2, b_fc2):
    global _NC_CACHE, _NC_KEY
    from concourse.bass_utils import run_bass_kernel_spmd

    x = np.asarray(x, np.float32)
    w_qkv = np.asarray(w_qkv, np.float32)
    ln1_g = np.asarray(ln1_g, np.float32)
    ln1_b = np.asarray(ln1_b, np.float32)
    ln2_g = np.asarray(ln2_g, np.float32)
    ln2_b = np.asarray(ln2_b, np.float32)

    ln1_affine = not (np.all(ln1_g == 1.0) and np.all(ln1_b == 0.0))
    ln2_affine = not (np.all(ln2_g == 1.0) and np.all(ln2_b == 0.0))

    wq = _reorder_qk(w_qkv[:, 0:C]) * WQ_S
    wk = _reorder_qk(w_qkv[:, C:2 * C]) * WQ_S
    wv = w_qkv[:, 2 * C:3 * C] * WV_S

    shared = {
        "wq_p": _pack_dr(wq).astype(FP8),
        "wk_p": _pack_dr(wk).astype(FP8),
        "wv_p": _pack_dr(wv).astype(FP8),
        "wp_p": _pack_dr(np.asarray(w_proj, np.float32) * WP_S).astype(FP8),
        "w1_p": _pack_dr(np.asarray(w_fc1, np.float32) * W1_S).astype(FP8),
        "w2_p": _pack_dr(np.asarray(w_fc2, np.float32) * W2_S).astype(FP8),
        "bp_s": np.asarray(b_proj, np.float32) * 512.0,
        "bf1_d": np.asarray(b_fc1, np.float32),
        "bf2_s": np.asarray(b_fc2, np.float32) * W2_S,
    }
    if ln1_affine:
        shared["ln1_g"] = ln1_g
        shared["ln1_b"] = ln1_b
    if ln2_affine:
        shared["ln2_g"] = ln2_g
        shared["ln2_b"] = ln2_b

    key = (ln1_affine, ln2_affine)
    if _NC_CACHE is None or _NC_KEY != key:
        _NC_CACHE = _build_bass(ln1_affine, ln2_affine)
        _NC_KEY = key

    in_maps = []
    for c in range(8):
        b, hh = c // 2, c % 2
        xbv = np.ascontiguousarray(np.roll(x[b], -hh * NO, axis=0))
        in_maps.append({"xb": xbv, **shared})

    res = run_bass_kernel_spmd(_NC_CACHE, in_maps, core_ids=list(range(8)))

    outp = np.empty((B, N, C), np.float32)
    for c in range(8):
        b, hh = c // 2, c % 2
        outp[b, hh * NO:(hh + 1) * NO, :] = res.results[c]["out"]
    return outp
